# revision 1
# baseline (speedup 1.0000x reference)
"""DistSAGEConv forward on 8 Trainium2 NeuronCores (Bass/Tile).

Math (matches the reference):
    h_neigh = segment_mean(local_feats[src], dst)            # [N, D]
    out     = relu(local_feats @ W_self.T + h_neigh @ W_neigh.T + b)

Distribution: nodes (and their incident dst edges) are sharded across the 8
cores, 6250 nodes each; the weights/bias are replicated; the full feature
table is replicated into every core's HBM so "remote neighbor features" are
just indirect-DMA gathers from the local copy (the halo exchange of the Dist
semantics collapses to a local gather because we receive full inputs).

Per core, per dst-tile of 128 nodes:
  1. dma_gather the tile's incident src rows (~2K x 2KB) into SBUF, in
     edge order (padded to chunks of 128).  Indices are int16, so the
     50000-row table is addressed as two 25000-row halves -> two gather
     calls per tile (edges pre-sorted into the two halves on host).
  2. For each 128-edge chunk, build S[e, j] = (dst_id[e] == j) * inv_deg[e]
     with one vector-engine tensor_scalar op (iota table == per-partition
     dst id, times per-partition 1/deg), then accumulate
     psum_h[128 dst, 512] += S.T @ G_chunk on the tensor engine (float32r
     streams fp32 at full rate for free dim >= 256).
  3. Transpose h via PE-transpose; accumulate
     psum_out = bias (K=1 matmul broadcast) + X_tile @ W_self.T + h @ W_neigh.T
     then ReLU on the scalar engine and DMA the tile out.

Edge bookkeeping (which edges belong to which tile/half, degrees, padding)
is integer preprocessing done on host with numpy; all floating-point math
happens on device.
"""

import numpy as np

from concourse import bass, bacc, mybir, tile
from concourse.bass_utils import run_bass_kernel_spmd

F32 = mybir.dt.float32
F32R = mybir.dt.float32r
I16 = mybir.dt.int16

N_NODES = 50000
N_EDGES = 800000
D = 512
NCORES = 8
NPC = N_NODES // NCORES          # 6250 nodes per core
P = 128                          # partitions / tile rows
NT = (NPC + P - 1) // P          # 49 dst tiles per core (last has 106 rows)
HALF = N_NODES // 2              # int16-addressable table half


class Plan:
    """Compile-time structure shared by all 8 cores (program is SPMD)."""

    def __init__(self, n_nodes, npc, half, tiles):
        self.n_nodes = n_nodes
        self.npc = npc
        self.half = half
        # tiles: list of (rows, cA, cB) -- cA/cB = 128-edge chunks for the
        # low/high table half, maxed across cores so one program fits all.
        self.tiles = tiles
        self.idx_off = []
        self.meta_off = []
        io = mo = 0
        for _, ca, cb in tiles:
            self.idx_off.append(io)
            self.meta_off.append(mo)
            io += (ca + cb) * 8          # int16 idx columns (16-wrap)
            mo += ca + cb                # one meta column per chunk
        self.sum_idx = io
        self.sum_ch = mo
        self.ch_max = max(ca + cb for _, ca, cb in tiles)

    def key(self):
        return (self.n_nodes, self.npc, self.half, tuple(self.tiles))


def _prepare(local_feats, src, dst, W_self, W_neigh, b,
             n_nodes=N_NODES, ncores=NCORES):
    """Host-side integer preprocessing -> (plan, in_maps)."""
    npc = n_nodes // ncores
    nt = (npc + P - 1) // P
    half = n_nodes // 2
    feats = np.ascontiguousarray(local_feats, dtype=np.float32)
    src = np.asarray(src).astype(np.int64)
    dst = np.asarray(dst).astype(np.int64)

    deg = np.bincount(dst, minlength=n_nodes).astype(np.float32)
    inv_node = (1.0 / np.maximum(deg, 1.0)).astype(np.float32)

    core_of = dst // npc
    local = dst - core_of * npc
    t_of = local // P
    r_of = (local % P).astype(np.float32)
    hi = (src >= half).astype(np.int64)
    key = (core_of * nt + t_of) * 2 + hi
    order = np.argsort(key, kind="stable")
    skey = key[order]
    ssrc = src[order]
    srid = r_of[order]
    sinv = inv_node[dst[order]]
    # segment boundaries for each (core, tile, half)
    bounds = np.searchsorted(skey, np.arange(ncores * nt * 2 + 1))

    def seg(c, t, h):
        k = (c * nt + t) * 2 + h
        return bounds[k], bounds[k + 1]

    # per-(t) chunk counts, maxed across cores
    tiles = []
    for t in range(nt):
        rows = min(P, npc - t * P)
        na = max(seg(c, t, 0)[1] - seg(c, t, 0)[0] for c in range(ncores))
        nb = max(seg(c, t, 1)[1] - seg(c, t, 1)[0] for c in range(ncores))
        ca = (na + P - 1) // P
        cb = (nb + P - 1) // P
        tiles.append((rows, ca, cb))
    plan = Plan(n_nodes, npc, half, tiles)

    # replicated constants
    wts = np.ascontiguousarray(
        W_self.T.astype(np.float32).reshape(4, P, D).transpose(1, 0, 2))
    wtn = np.ascontiguousarray(
        W_neigh.T.astype(np.float32).reshape(4, P, D).transpose(1, 0, 2))
    bias = np.ascontiguousarray(b.astype(np.float32).reshape(1, D))
    ones = np.ones((1, P), dtype=np.float32)
    ident = np.eye(P, dtype=np.float32)
    iota = np.tile(np.arange(P, dtype=np.float32), (P, 1))
    iota = np.ascontiguousarray(iota)

    in_maps = []
    for c in range(ncores):
        idx_cols = []
        rid_cols = []
        inv_cols = []
        for t in range(nt):
            rows, ca, cb = plan.tiles[t]
            for h, cn in ((0, ca), (1, cb)):
                lo, hiq = seg(c, t, h)
                n = hiq - lo
                npad = cn * P
                iv = np.zeros(npad, dtype=np.int16)
                iv[:n] = (ssrc[lo:hiq] - h * half).astype(np.int16)
                # idx wrap: i -> [i%16, i//16], replicated to 128 partitions
                m = iv.reshape(npad // 16, 16).T
                idx_cols.append(np.tile(m, (8, 1)))
                rv = np.full(npad, 255.0, dtype=np.float32)
                rv[:n] = srid[lo:hiq]
                vv = np.zeros(npad, dtype=np.float32)
                vv[:n] = sinv[lo:hiq]
                # meta wrap: i -> [i%128, i//128]
                rid_cols.append(rv.reshape(cn, P).T)
                inv_cols.append(vv.reshape(cn, P).T)
        eidx = np.ascontiguousarray(np.concatenate(idx_cols, axis=1))
        edst = np.ascontiguousarray(np.concatenate(rid_cols, axis=1))
        einv = np.ascontiguousarray(np.concatenate(inv_cols, axis=1))

        # self-chunk, transposed + tiled: xt[t, p, f, j] = Xc[t*128+j, f*128+p]
        xc = np.zeros((nt * P, D), dtype=np.float32)
        xc[:npc] = feats[c * npc:(c + 1) * npc]
        xt = np.ascontiguousarray(
            xc.reshape(nt, P, 4, P).transpose(0, 3, 2, 1))

        in_maps.append({
            "feats": feats,
            "xt": xt,
            "wts": wts,
            "wtn": wtn,
            "bias": bias,
            "ones": ones,
            "ident": ident,
            "iota": iota,
            "eidx": eidx,
            "edst": edst,
            "einv": einv,
        })
    return plan, in_maps


def build(plan, mode="full"):
    """Build + compile the SPMD Bass program for one core.

    mode: debug switch -- "full", "nogather" (memset G instead of gather),
    "notrans" (skip PE transposes; hT = h copy), "noagg" (skip S matmuls).
    """
    nc = bacc.Bacc("TRN2", target_bir_lowering=False, debug=False,
                   enable_asserts=False, num_devices=NCORES)
    n_nodes, npc, half = plan.n_nodes, plan.npc, plan.half
    nt = len(plan.tiles)

    feats = nc.dram_tensor("feats", [n_nodes, D], F32R, kind="ExternalInput")
    xt = nc.dram_tensor("xt", [nt, P, 4, P], F32R, kind="ExternalInput")
    wts = nc.dram_tensor("wts", [P, 4, D], F32R, kind="ExternalInput")
    wtn = nc.dram_tensor("wtn", [P, 4, D], F32R, kind="ExternalInput")
    bias = nc.dram_tensor("bias", [1, D], F32R, kind="ExternalInput")
    ones = nc.dram_tensor("ones", [1, P], F32R, kind="ExternalInput")
    ident = nc.dram_tensor("ident", [P, P], F32R, kind="ExternalInput")
    iota = nc.dram_tensor("iota", [P, P], F32, kind="ExternalInput")
    eidx = nc.dram_tensor("eidx", [P, plan.sum_idx], I16, kind="ExternalInput")
    edst = nc.dram_tensor("edst", [P, plan.sum_ch], F32, kind="ExternalInput")
    einv = nc.dram_tensor("einv", [P, plan.sum_ch], F32, kind="ExternalInput")
    out = nc.dram_tensor("out", [npc, D], F32, kind="ExternalOutput")

    AF = mybir.ActivationFunctionType
    OP = mybir.AluOpType

    with tile.TileContext(nc) as tc:
        with (
            tc.tile_pool(name="const", bufs=1) as cpool,
            tc.tile_pool(name="g", bufs=2) as gpool,
            tc.tile_pool(name="s", bufs=4) as spool,
            tc.tile_pool(name="x", bufs=2) as xpool,
            tc.tile_pool(name="h", bufs=2) as hpool,
            tc.tile_pool(name="ht", bufs=2) as htpool,
            tc.tile_pool(name="o", bufs=2) as opool,
            tc.tile_pool(name="ph", bufs=2, space="PSUM") as phpool,
            tc.tile_pool(name="ptr", bufs=2, space="PSUM") as ptrpool,
            tc.tile_pool(name="po", bufs=2, space="PSUM") as popool,
        ):
            # resident constants
            wts_s = cpool.tile([P, 4, D], F32R, tag="wts")
            nc.sync.dma_start(wts_s[:], wts[:])
            wtn_s = cpool.tile([P, 4, D], F32R, tag="wtn")
            nc.sync.dma_start(wtn_s[:], wtn[:])
            bias_s = cpool.tile([1, D], F32R, tag="bias")
            nc.sync.dma_start(bias_s[:], bias[:])
            ones_s = cpool.tile([1, P], F32R, tag="ones")
            nc.sync.dma_start(ones_s[:], ones[:])
            ident_s = cpool.tile([P, P], F32R, tag="ident")
            nc.sync.dma_start(ident_s[:], ident[:])
            iota_s = cpool.tile([P, P], F32, tag="iota")
            nc.sync.dma_start(iota_s[:], iota[:])
            idx_s = cpool.tile([P, plan.sum_idx], I16, tag="eidx")
            nc.sync.dma_start(idx_s[:], eidx[:])
            dst_s = cpool.tile([P, plan.sum_ch], F32, tag="edst")
            nc.sync.dma_start(dst_s[:], edst[:])
            inv_s = cpool.tile([P, plan.sum_ch], F32, tag="einv")
            nc.sync.dma_start(inv_s[:], einv[:])

            feats_a = feats[0:half, :]
            feats_b = feats[half:n_nodes, :]

            for t in range(nt):
                rows, ca, cb = plan.tiles[t]
                ch = ca + cb
                io = plan.idx_off[t]
                mo = plan.meta_off[t]

                g = gpool.tile([P, plan.ch_max, D], F32R, tag="g")
                if mode == "nogather":
                    nc.gpsimd.memset(g[:], 0.0)
                else:
                    # >=1024 indices in one dma_gather wedges the device
                    # (NRT_EXEC_UNIT_UNRECOVERABLE); split into <=768-idx
                    # sub-calls.
                    GMAX = 6
                    for base, cn, src_ap in ((0, ca, feats_a),
                                             (ca, cb, feats_b)):
                        for c0 in range(0, cn, GMAX):
                            cw = min(GMAX, cn - c0)
                            nc.gpsimd.dma_gather(
                                g[:, base + c0:base + c0 + cw, :], src_ap,
                                idx_s[:, io + (base + c0) * 8:
                                      io + (base + c0 + cw) * 8],
                                cw * P, cw * P, D)

                xt_t = xpool.tile([P, 4, P], F32R, tag="x")
                nc.sync.dma_start(xt_t[:], xt[t])

                # aggregation: psum_h[dst, feat] += S_c.T @ G_c
                ph = phpool.tile([P, D], F32, tag="ph")
                if mode == "noagg":
                    nc.vector.memset(ph[:], 0.0)
                else:
                    for c in range(ch):
                        s = spool.tile([P, P], F32R, tag="s")
                        nc.vector.tensor_scalar(
                            s[:], iota_s[:],
                            dst_s[:, mo + c:mo + c + 1],
                            inv_s[:, mo + c:mo + c + 1],
                            op0=OP.is_equal, op1=OP.mult)
                        nc.tensor.matmul(
                            ph[:], s[:], g[:, c, :],
                            start=(c == 0), stop=(c == ch - 1))

                h = hpool.tile([P, D], F32R, tag="h")
                nc.vector.tensor_copy(h[:], ph[:])

                ht = htpool.tile([P, 4, P], F32R, tag="ht")
                if mode == "notrans":
                    nc.vector.tensor_copy(ht[:], h[:])
                else:
                    # transpose h -> hT as plain matmuls against identity
                    # (h_f.T @ I); avoids is_transpose PE-mode switches,
                    # which hang the device when mixed with f32r matmuls.
                    ptr = ptrpool.tile([P, 4, P], F32, tag="ptr")
                    for f in range(4):
                        nc.tensor.matmul(
                            ptr[:, f, :], h[:, f * P:(f + 1) * P], ident_s[:],
                            start=True, stop=True)
                    nc.vector.tensor_copy(ht[:], ptr[:])

                # out = relu(bias + X @ Wself.T + h @ Wneigh.T)
                po = popool.tile([P, D], F32, tag="po")
                nc.tensor.matmul(po[:], ones_s[:], bias_s[:],
                                 start=True, stop=False)
                for f in range(4):
                    nc.tensor.matmul(po[:], xt_t[:, f, :], wts_s[:, f, :],
                                     start=False, stop=False)
                    nc.tensor.matmul(po[:], ht[:, f, :], wtn_s[:, f, :],
                                     start=False, stop=(f == 3))

                o = opool.tile([P, D], F32, tag="o")
                nc.scalar.activation(o[:], po[:], AF.Relu)
                nc.sync.dma_start(out[t * P:t * P + rows, :], o[:rows, :])

    nc.compile()
    return nc


_cache = {}


def _get_nc(plan):
    k = plan.key()
    if k not in _cache:
        _cache[k] = build(plan)
    return _cache[k]


def kernel(local_feats, src, dst, layer=None, W_self=None, W_neigh=None,
           b=None, **_unused):
    plan, in_maps = _prepare(local_feats, src, dst, W_self, W_neigh, b)
    nc = _get_nc(plan)
    res = run_bass_kernel_spmd(nc, in_maps, core_ids=list(range(NCORES)))
    return np.concatenate([res.results[c]["out"] for c in range(NCORES)],
                          axis=0)

